# revision 5
# baseline (speedup 1.0000x reference)
"""AffineCoupling (NICE) forward on 8 Trainium2 NeuronCores.

Data-parallel over the batch: each core gets a 1024-row slice of x and
runs the full 6-layer MLP (512->2048->2048x4->1024) with replicated
weights, then the coupling epilogue (tanh / exp / scale+shift /
log-det) on chip.

Layout: activations are kept feature-major (hT[feat, batch]) so the
weight matrices, stored [in, out], are directly the stationary matmul
operand and the chain needs no per-layer transposes.  Matmuls run in
fp32r (FP22-precision fp32, full PE rate at N=512).  The even/odd
feature split of the coupling is folded into the M-tiling of the last
layer, so tanh/exp/shift operate on contiguous tiles.
"""

import sys

for _p in ("/opt/trn_rl_repo",):
    if _p not in sys.path:
        sys.path.insert(0, _p)

from contextlib import ExitStack

import numpy as np

import concourse.bass as bass
import concourse.tile as tile
from concourse import bacc, mybir
from concourse.bass_utils import run_bass_kernel_spmd
from concourse.masks import make_identity

P = 128
NCORES = 8
B = 8192  # full batch
W = 1024  # feature width
BC = B // NCORES  # batch rows per core (1024)
NT = 512  # matmul moving-dim tile (max for fp32, = 1 PSUM bank)
MID = 2048
F32 = mybir.dt.float32
F32R = mybir.dt.float32r
AF = mybir.ActivationFunctionType

# (in_dim, out_dim) per layer
L_DIMS = [(W // 2, MID)] + [(MID, MID)] * 4 + [(MID, W)]


def _build_nc():
    nc = bacc.Bacc(trn_type="TRN2", target_bir_lowering=False, debug=False)

    x_d = nc.dram_tensor("x", [BC, W], F32, kind="ExternalInput").ap()
    ldj_d = nc.dram_tensor("log_det_J", [BC], F32, kind="ExternalInput").ap()
    w_d = []
    b_d = []
    for i, (di, do) in enumerate(L_DIMS):
        w_d.append(nc.dram_tensor(f"w{i}", [di, do], F32, kind="ExternalInput").ap())
        b_d.append(nc.dram_tensor(f"b{i}", [do], F32, kind="ExternalInput").ap())
    y_d = nc.dram_tensor("y", [BC, W], F32, kind="ExternalOutput").ap()
    ld_d = nc.dram_tensor("log_det", [BC], F32, kind="ExternalOutput").ap()

    with tile.TileContext(nc) as tc, ExitStack() as ctx:
        const = ctx.enter_context(tc.tile_pool(name="const", bufs=1))
        xt_pool = ctx.enter_context(tc.tile_pool(name="xt", bufs=1))
        xin_pool = ctx.enter_context(tc.tile_pool(name="xin", bufs=4))
        h_pool = ctx.enter_context(tc.tile_pool(name="h", bufs=2))
        w_pool = ctx.enter_context(tc.tile_pool(name="w", bufs=2))
        mm_ps = ctx.enter_context(tc.tile_pool(name="mmps", bufs=4, space="PSUM"))
        tr_ps = ctx.enter_context(tc.tile_pool(name="trps", bufs=2, space="PSUM"))
        ld_ps = ctx.enter_context(tc.tile_pool(name="ldps", bufs=2, space="PSUM"))

        ident_f = const.tile([P, P], F32, tag="identf")
        make_identity(nc, ident_f)
        ident = const.tile([P, P], F32R, tag="ident")
        nc.vector.tensor_copy(ident[:], ident_f[:])
        ones_f = const.tile([P, 1], F32, tag="onesf")
        nc.gpsimd.memset(ones_f[:], 1.0)
        ones = const.tile([P, 1], F32R, tag="ones")
        nc.vector.tensor_copy(ones[:], ones_f[:])

        # Biases, feature-major: column m holds features m*128..m*128+127.
        bias_sb = []
        for l, (_, do) in enumerate(L_DIMS[:5]):
            bt = const.tile([P, do // P], F32, tag=f"bias{l}")
            nc.sync.dma_start(bt[:], b_d[l].rearrange("(mo p) -> p mo", p=P))
            bias_sb.append(bt)
        b5r = b_d[5].rearrange("(mo p two) -> p mo two", p=P, two=2)
        b5e = const.tile([P, 4], F32, tag="b5e")
        nc.sync.dma_start(b5e[:], b5r[:, :, 0])
        b5o = const.tile([P, 4], F32, tag="b5o")
        nc.sync.dma_start(b5o[:], b5r[:, :, 1])
        ldj_sb = const.tile([1, BC], F32, tag="ldj")
        nc.sync.dma_start(ldj_sb[:], ldj_d.unsqueeze(0))

        # ---- Stage A: split-transpose x into x1T (even cols) / x2T (odd) ----
        x1T = xt_pool.tile([P, 4, BC], F32R)  # [feat, chunk, batch]
        x2T = xt_pool.tile([P, 4, BC], F32)
        for bb in range(BC // P):
            for s in range(4):  # 256-col spans of x
                xc = xin_pool.tile([P, 256], F32R, tag="xblk")
                nc.sync.dma_start(
                    xc[:],
                    x_d[bb * P : (bb + 1) * P, s * 256 : (s + 1) * 256].bitcast(F32R),
                )
                pe = tr_ps.tile([P, P], F32R, tag="tr")
                nc.tensor.transpose(pe[:], xc[:, 0:256:2], ident[:])
                nc.vector.tensor_copy(x1T[:, s, bb * P : (bb + 1) * P], pe[:])
                po = tr_ps.tile([P, P], F32R, tag="tr")
                nc.tensor.transpose(po[:], xc[:, 1:256:2], ident[:])
                nc.vector.tensor_copy(x2T[:, s, bb * P : (bb + 1) * P], po[:])

        # ---- MLP chain, feature-major ----
        def load_w_tile(wr, col0, ncols, ko):
            """DMA one stationary-weight tile, split for DMA-queue parallelism."""
            wt = w_pool.tile([P, 16, 256], F32R, tag="w")
            nchunk = 4 if ko >= 8 else 2
            step = ko // nchunk
            for c in range(nchunk):
                nc.sync.dma_start(
                    wt[:, c * step : (c + 1) * step, :ncols],
                    wr[:, c * step : (c + 1) * step, col0 : col0 + ncols].bitcast(F32R),
                )
            return wt

        cur = x1T
        h5 = None
        for l in range(6):
            di, do = L_DIMS[l]
            ko = di // P
            wr = w_d[l].rearrange("(ko p) m -> p ko m", p=P)
            if l < 5:
                nxt = h_pool.tile([P, 16, BC], F32R, tag="h")
                for m in range(do // P):
                    wt = load_w_tile(wr, m * P, P, ko)
                    for n in range(BC // NT):
                        acc = mm_ps.tile([P, NT], F32, tag="acc")
                        for k in range(ko):
                            nc.tensor.matmul(
                                acc[:],
                                wt[:, k, :P],
                                cur[:, k, n * NT : (n + 1) * NT],
                                start=(k == 0),
                                stop=(k == ko - 1),
                            )
                        nc.scalar.activation(
                            nxt[:, m, n * NT : (n + 1) * NT],
                            acc[:],
                            AF.Relu,
                            bias=bias_sb[l][:, m : m + 1],
                        )
                cur = nxt
            else:
                # Last layer: even out-features -> tanh (log-scale),
                # odd out-features -> identity+bias (shift).
                h5 = h_pool.tile([P, 16, BC], F32R, tag="h")
                b1tT = h5[:, 0:4]  # tanh output, even features
                b2tT = h5[:, 4:8]  # shift, odd features
                for m in range(4):  # 256-wide contiguous spans of w5 cols
                    wt = load_w_tile(wr, m * 256, 256, ko)
                    for n in range(BC // NT):
                        nsl = slice(n * NT, (n + 1) * NT)
                        acc_e = mm_ps.tile([P, NT], F32, tag="acc")
                        for k in range(ko):
                            nc.tensor.matmul(
                                acc_e[:],
                                wt[:, k, 0:256:2],
                                cur[:, k, nsl],
                                start=(k == 0),
                                stop=(k == ko - 1),
                            )
                        nc.scalar.activation(
                            b1tT[:, m, nsl],
                            acc_e[:],
                            AF.Tanh,
                            bias=b5e[:, m : m + 1],
                        )
                        acc_o = mm_ps.tile([P, NT], F32, tag="acc")
                        for k in range(ko):
                            nc.tensor.matmul(
                                acc_o[:],
                                wt[:, k, 1:256:2],
                                cur[:, k, nsl],
                                start=(k == 0),
                                stop=(k == ko - 1),
                            )
                        nc.scalar.activation(
                            b2tT[:, m, nsl],
                            acc_o[:],
                            AF.Identity,
                            bias=b5o[:, m : m + 1],
                        )

        assert h5 is not None
        b1tT = h5[:, 0:4]
        b2tT = h5[:, 4:8]
        expb = h5[:, 8:12]
        y2T = h5[:, 12:16]
        b1tT_r = b1tT

        # ---- log_det = log_det_J + sum_f tanh(...) (partition reduce via PE) ----
        ld_sb = const.tile([1, BC], F32, tag="ldout")
        for n in range(BC // NT):
            nsl = slice(n * NT, (n + 1) * NT)
            lacc = ld_ps.tile([1, NT], F32, tag="ld")
            for m in range(4):
                nc.tensor.matmul(
                    lacc[:], ones[:], b1tT_r[:, m, nsl], start=(m == 0), stop=(m == 3)
                )
            nc.vector.tensor_add(ld_sb[:, nsl], lacc[:], ldj_sb[:, nsl])
        nc.sync.dma_start(ld_d.unsqueeze(0), ld_sb[:])

        # ---- y2 = x2 * exp(b1t) + b2t (feature-major) ----
        for c in range(4):
            nc.scalar.activation(expb[:, c], b1tT_r[:, c], AF.Exp)
            nc.vector.tensor_mul(y2T[:, c], expb[:, c], x2T[:, c])
            nc.vector.tensor_add(y2T[:, c], y2T[:, c], b2tT[:, c])

        # ---- Stage D: transpose back + interleave into y ----
        y_combo = h_pool.tile([P, 16, BC], F32, tag="h")
        y_sb = y_combo[:, 0:8]  # [P, batch_block, 1024] batch-major
        for bb in range(BC // P):
            bsl = slice(bb * P, (bb + 1) * P)
            for s in range(4):
                pt = tr_ps.tile([P, P], F32R, tag="tr")
                nc.tensor.transpose(pt[:], y2T[:, s, bsl], ident[:])
                nc.vector.tensor_copy(
                    y_sb[:, bb, 2 * s * P + 1 : 2 * (s + 1) * P : 2], pt[:]
                )
                pt2 = tr_ps.tile([P, P], F32R, tag="tr")
                nc.tensor.transpose(pt2[:], x1T[:, s, bsl], ident[:])
                nc.vector.tensor_copy(
                    y_sb[:, bb, 2 * s * P : 2 * (s + 1) * P : 2], pt2[:]
                )
        for bb in range(BC // P):
            nc.sync.dma_start(y_d[bb * P : (bb + 1) * P, :], y_sb[:, bb])

    nc.compile()
    return nc


_NC_CACHE = None


def _get_nc():
    global _NC_CACHE
    if _NC_CACHE is None:
        _NC_CACHE = _build_nc()
    return _NC_CACHE


def kernel(**inputs):
    x = np.ascontiguousarray(np.asarray(inputs["x"], dtype=np.float32))
    ldj = np.ascontiguousarray(np.asarray(inputs["log_det_J"], dtype=np.float32))
    weights = {}
    for i in range(6):
        weights[f"w{i}"] = np.ascontiguousarray(
            np.asarray(inputs[f"w{i}"], dtype=np.float32)
        )
        weights[f"b{i}"] = np.ascontiguousarray(
            np.asarray(inputs[f"b{i}"], dtype=np.float32)
        )

    nc = _get_nc()
    in_maps = []
    for c in range(NCORES):
        m = {"x": x[c * BC : (c + 1) * BC], "log_det_J": ldj[c * BC : (c + 1) * BC]}
        m.update(weights)
        in_maps.append(m)
    res = run_bass_kernel_spmd(nc, in_maps, list(range(NCORES)))

    y = np.concatenate([res.results[c]["y"] for c in range(NCORES)], axis=0)
    ld = np.concatenate([res.results[c]["log_det"] for c in range(NCORES)], axis=0)
    return (y, ld)


# revision 6
# speedup vs baseline: 1.0310x; 1.0310x over previous
"""AffineCoupling (NICE) forward on 8 Trainium2 NeuronCores.

Data-parallel over the batch: each core gets a 1024-row slice of x and
runs the full 6-layer MLP (512->2048->2048x4->1024) with replicated
weights, then the coupling epilogue (tanh / exp / scale+shift /
log-det) on chip.

Layout: activations are kept feature-major (hT[feat, batch]) so the
weight matrices, stored [in, out], are directly the stationary matmul
operand and the chain needs no per-layer transposes.  Matmuls run in
fp32r (FP22-precision fp32, full PE rate at N=512).  The even/odd
feature split of the coupling is folded into the M-tiling of the last
layer, so tanh/exp/shift operate on contiguous tiles.
"""

import sys

for _p in ("/opt/trn_rl_repo",):
    if _p not in sys.path:
        sys.path.insert(0, _p)

from contextlib import ExitStack

import numpy as np

import concourse.bass as bass
import concourse.tile as tile
from concourse import bacc, mybir
from concourse.bass_utils import run_bass_kernel_spmd
from concourse.masks import make_identity

P = 128
NCORES = 8
B = 8192  # full batch
W = 1024  # feature width
BC = B // NCORES  # batch rows per core (1024)
NT = 512  # matmul moving-dim tile (max for fp32, = 1 PSUM bank)
MID = 2048
F32 = mybir.dt.float32
F32R = mybir.dt.float32r
AF = mybir.ActivationFunctionType

# (in_dim, out_dim) per layer
L_DIMS = [(W // 2, MID)] + [(MID, MID)] * 4 + [(MID, W)]


def _build_nc():
    nc = bacc.Bacc(trn_type="TRN2", target_bir_lowering=False, debug=False)

    x_d = nc.dram_tensor("x", [BC, W], F32, kind="ExternalInput").ap()
    ldj_d = nc.dram_tensor("log_det_J", [BC], F32, kind="ExternalInput").ap()
    w_d = []
    b_d = []
    for i, (di, do) in enumerate(L_DIMS):
        w_d.append(nc.dram_tensor(f"w{i}", [di, do], F32, kind="ExternalInput").ap())
        b_d.append(nc.dram_tensor(f"b{i}", [do], F32, kind="ExternalInput").ap())
    y_d = nc.dram_tensor("y", [BC, W], F32, kind="ExternalOutput").ap()
    ld_d = nc.dram_tensor("log_det", [BC], F32, kind="ExternalOutput").ap()

    with tile.TileContext(nc) as tc, ExitStack() as ctx:
        const = ctx.enter_context(tc.tile_pool(name="const", bufs=1))
        xt_pool = ctx.enter_context(tc.tile_pool(name="xt", bufs=1))
        xin_pool = ctx.enter_context(tc.tile_pool(name="xin", bufs=6))
        h_pool = ctx.enter_context(tc.tile_pool(name="h", bufs=2))
        w_pool = ctx.enter_context(tc.tile_pool(name="w", bufs=2))
        mm_ps = ctx.enter_context(tc.tile_pool(name="mmps", bufs=4, space="PSUM"))
        tr_ps = ctx.enter_context(tc.tile_pool(name="trps", bufs=2, space="PSUM"))
        ld_ps = ctx.enter_context(tc.tile_pool(name="ldps", bufs=2, space="PSUM"))

        ident_f = const.tile([P, P], F32, tag="identf")
        make_identity(nc, ident_f)
        ident = const.tile([P, P], F32R, tag="ident")
        nc.vector.tensor_copy(ident[:], ident_f[:])
        ones_f = const.tile([P, 1], F32, tag="onesf")
        nc.gpsimd.memset(ones_f[:], 1.0)
        ones = const.tile([P, 1], F32R, tag="ones")
        nc.vector.tensor_copy(ones[:], ones_f[:])

        # Biases, feature-major: column m holds features m*128..m*128+127.
        bias_sb = []
        for l, (_, do) in enumerate(L_DIMS[:5]):
            bt = const.tile([P, do // P], F32, tag=f"bias{l}")
            nc.sync.dma_start(bt[:], b_d[l].rearrange("(mo p) -> p mo", p=P))
            bias_sb.append(bt)
        b5r = b_d[5].rearrange("(mo p two) -> p mo two", p=P, two=2)
        b5e = const.tile([P, 4], F32, tag="b5e")
        nc.sync.dma_start(b5e[:], b5r[:, :, 0])
        b5o = const.tile([P, 4], F32, tag="b5o")
        nc.sync.dma_start(b5o[:], b5r[:, :, 1])
        ldj_sb = const.tile([1, BC], F32, tag="ldj")
        nc.sync.dma_start(ldj_sb[:], ldj_d.unsqueeze(0))

        # ---- Stage A: split-transpose x into x1T (even cols) / x2T (odd) ----
        x1T = xt_pool.tile([P, 4, BC], F32R)  # [feat, chunk, batch]
        x2T = xt_pool.tile([P, 4, BC], F32)
        for bb in range(BC // P):
            for s in range(4):  # 256-col spans of x
                xc = xin_pool.tile([P, 256], F32R, tag="xblk")
                nc.sync.dma_start(
                    xc[:],
                    x_d[bb * P : (bb + 1) * P, s * 256 : (s + 1) * 256].bitcast(F32R),
                )
                pe = tr_ps.tile([P, P], F32R, tag="tr")
                nc.tensor.transpose(pe[:], xc[:, 0:256:2], ident[:])
                nc.vector.tensor_copy(x1T[:, s, bb * P : (bb + 1) * P], pe[:])
                po = tr_ps.tile([P, P], F32R, tag="tr")
                nc.tensor.transpose(po[:], xc[:, 1:256:2], ident[:])
                nc.vector.tensor_copy(x2T[:, s, bb * P : (bb + 1) * P], po[:])

        # ---- MLP chain, feature-major ----
        def load_w_tile(wr, col0, ncols, ko):
            """DMA one stationary-weight tile, split for DMA-queue parallelism."""
            wt = w_pool.tile([P, 16, 256], F32R, tag="w")
            nchunk = 4 if ko >= 8 else 2
            step = ko // nchunk
            for c in range(nchunk):
                nc.sync.dma_start(
                    wt[:, c * step : (c + 1) * step, :ncols],
                    wr[:, c * step : (c + 1) * step, col0 : col0 + ncols].bitcast(F32R),
                )
            return wt

        cur = x1T
        h5 = None
        for l in range(6):
            di, do = L_DIMS[l]
            ko = di // P
            wr = w_d[l].rearrange("(ko p) m -> p ko m", p=P)
            if l < 5:
                nxt = h_pool.tile([P, 16, BC], F32R, tag="h")
                for mp in range(do // 256):  # paired m-tiles: 1KB DMA runs
                    wt = load_w_tile(wr, mp * 256, 256, ko)
                    for ms in range(2):
                        m = mp * 2 + ms
                        for n in range(BC // NT):
                            acc = mm_ps.tile([P, NT], F32, tag="acc")
                            for k in range(ko):
                                nc.tensor.matmul(
                                    acc[:],
                                    wt[:, k, ms * P : (ms + 1) * P],
                                    cur[:, k, n * NT : (n + 1) * NT],
                                    start=(k == 0),
                                    stop=(k == ko - 1),
                                )
                            nc.scalar.activation(
                                nxt[:, m, n * NT : (n + 1) * NT],
                                acc[:],
                                AF.Relu,
                                bias=bias_sb[l][:, m : m + 1],
                            )
                cur = nxt
            else:
                # Last layer: even out-features -> tanh (log-scale),
                # odd out-features -> identity+bias (shift).
                h5 = h_pool.tile([P, 16, BC], F32R, tag="h")
                b1tT = h5[:, 0:4]  # tanh output, even features
                b2tT = h5[:, 4:8]  # shift, odd features
                for m in range(4):  # 256-wide contiguous spans of w5 cols
                    wt = load_w_tile(wr, m * 256, 256, ko)
                    for n in range(BC // NT):
                        nsl = slice(n * NT, (n + 1) * NT)
                        acc_e = mm_ps.tile([P, NT], F32, tag="acc")
                        for k in range(ko):
                            nc.tensor.matmul(
                                acc_e[:],
                                wt[:, k, 0:256:2],
                                cur[:, k, nsl],
                                start=(k == 0),
                                stop=(k == ko - 1),
                            )
                        nc.scalar.activation(
                            b1tT[:, m, nsl],
                            acc_e[:],
                            AF.Tanh,
                            bias=b5e[:, m : m + 1],
                        )
                        acc_o = mm_ps.tile([P, NT], F32, tag="acc")
                        for k in range(ko):
                            nc.tensor.matmul(
                                acc_o[:],
                                wt[:, k, 1:256:2],
                                cur[:, k, nsl],
                                start=(k == 0),
                                stop=(k == ko - 1),
                            )
                        nc.scalar.activation(
                            b2tT[:, m, nsl],
                            acc_o[:],
                            AF.Identity,
                            bias=b5o[:, m : m + 1],
                        )

        assert h5 is not None
        b1tT = h5[:, 0:4]
        b2tT = h5[:, 4:8]
        expb = h5[:, 8:12]
        y2T = h5[:, 12:16]
        b1tT_r = b1tT

        # ---- log_det = log_det_J + sum_f tanh(...) (partition reduce via PE) ----
        ld_sb = const.tile([1, BC], F32, tag="ldout")
        for n in range(BC // NT):
            nsl = slice(n * NT, (n + 1) * NT)
            lacc = ld_ps.tile([1, NT], F32, tag="ld")
            for m in range(4):
                nc.tensor.matmul(
                    lacc[:], ones[:], b1tT_r[:, m, nsl], start=(m == 0), stop=(m == 3)
                )
            nc.vector.tensor_add(ld_sb[:, nsl], lacc[:], ldj_sb[:, nsl])
        nc.sync.dma_start(ld_d.unsqueeze(0), ld_sb[:])

        # ---- y2 = x2 * exp(b1t) + b2t (feature-major) ----
        for c in range(4):
            nc.scalar.activation(expb[:, c], b1tT_r[:, c], AF.Exp)
            nc.vector.tensor_mul(y2T[:, c], expb[:, c], x2T[:, c])
            nc.vector.tensor_add(y2T[:, c], y2T[:, c], b2tT[:, c])

        # ---- Stage D: transpose back + interleave into y ----
        y_combo = h_pool.tile([P, 16, BC], F32, tag="h")
        y_sb = y_combo[:, 0:8]  # [P, batch_block, 1024] batch-major
        for bb in range(BC // P):
            bsl = slice(bb * P, (bb + 1) * P)
            for s in range(4):
                pt = tr_ps.tile([P, P], F32R, tag="tr")
                nc.tensor.transpose(pt[:], y2T[:, s, bsl], ident[:])
                nc.vector.tensor_copy(
                    y_sb[:, bb, 2 * s * P + 1 : 2 * (s + 1) * P : 2], pt[:]
                )
                pt2 = tr_ps.tile([P, P], F32R, tag="tr")
                nc.tensor.transpose(pt2[:], x1T[:, s, bsl], ident[:])
                nc.vector.tensor_copy(
                    y_sb[:, bb, 2 * s * P : 2 * (s + 1) * P : 2], pt2[:]
                )
        for bb in range(BC // P):
            nc.sync.dma_start(y_d[bb * P : (bb + 1) * P, :], y_sb[:, bb])

    nc.compile()
    return nc


_NC_CACHE = None


def _get_nc():
    global _NC_CACHE
    if _NC_CACHE is None:
        _NC_CACHE = _build_nc()
    return _NC_CACHE


def kernel(**inputs):
    x = np.ascontiguousarray(np.asarray(inputs["x"], dtype=np.float32))
    ldj = np.ascontiguousarray(np.asarray(inputs["log_det_J"], dtype=np.float32))
    weights = {}
    for i in range(6):
        weights[f"w{i}"] = np.ascontiguousarray(
            np.asarray(inputs[f"w{i}"], dtype=np.float32)
        )
        weights[f"b{i}"] = np.ascontiguousarray(
            np.asarray(inputs[f"b{i}"], dtype=np.float32)
        )

    nc = _get_nc()
    in_maps = []
    for c in range(NCORES):
        m = {"x": x[c * BC : (c + 1) * BC], "log_det_J": ldj[c * BC : (c + 1) * BC]}
        m.update(weights)
        in_maps.append(m)
    res = run_bass_kernel_spmd(nc, in_maps, list(range(NCORES)))

    y = np.concatenate([res.results[c]["y"] for c in range(NCORES)], axis=0)
    ld = np.concatenate([res.results[c]["log_det"] for c in range(NCORES)], axis=0)
    return (y, ld)
